# revision 54
# baseline (speedup 1.0000x reference)
"""Trainium2 Bass kernel for AtomicDifferentiatedDense (moe_routing), v3.

Computation (full shapes):
    x            [2048, 128, 128] f32
    atom_numbers [2048, 128]      i32
    W            [4, 128, 128]    f32
    b            [4, 128]         f32   (zeros for this problem)
    atom_cases   [4]              i32
    out[b,a,o] = relu(x[b,a,:] @ W[e] + b[e]) where atom_numbers[b,a] ==
    atom_cases[e], else 0.

v3 design (vs the v2 all-token masked formulation):
  Each token matches at most ONE expert (atom_cases are distinct), and
  ~5/9 of tokens match none (their output is exactly zero).  So:

  - Host: bucket token indices by expert (np equality + nonzero), pad
    each expert's global list to 8*cap_e slots, split across the 8
    cores.  Gather the matched x rows, cast to bf16, and TRANSPOSE on
    host so the device sees xt[ci, tok] — the exact rhs layout the PE
    wants.  Unmatched tokens never touch the device; their output rows
    stay zero.
  - Device (per core): for each expert e, DMA [128, cap_e] slabs of
    pre-transposed tokens, one N<=512 matmul per token chunk
    (lhsT = W_e [ci, o] stationary, rhs = xt chunk [ci, tok] moving),
    relu fused into the PSUM->SBUF bounce (DVE/ACT), DMA the [o, tok]
    result back.  No transposes, no masks, no atom_numbers on device.
  - Host: upcast, transpose back, scatter rows into np.zeros output,
    dropping padding slots.

  Device traffic per core: 2 * sum(cap_e) * 128 * 2B ~ 8.4 MB
  (cap=4096) vs 16.8 MB for v2; PE work drops 16x.

  Shipped config (DEFAULT_BUILD_KWARGS): build_nc_dual with warm_mm=10 —
  two independent lanes (experts 0,1 relu-bounce on DVE; experts 2,3 on
  ACT) with per-lane xb/PSUM/ob pools, interleaved at group granularity
  so both drain engines run concurrently (PSUM reads are 1 elem/cycle
  per engine; a single engine would serialize ~19us of bounces), plus 10
  dummy matmuls at the top of the pass so the PE's p-state ramp (1.2 ->
  2.4 GHz after ~3us of continuous activity) completes during the first
  input DMA.  Measured: 28138 ns/pass vs 207744 ns baseline (~7.4x).
  Also tried and rejected on measurement: head/tail edge-split blocks,
  out-DMA per group, gpsimd/scalar-ring DMA placement, exact (smaller)
  capacities, 4-group drains, 1024-wide matmuls (compiler crash).
"""

import contextlib
import sys

import numpy as np

import concourse.bacc as bacc
import concourse.mybir as mybir
import concourse.tile as tile
from concourse.bass_utils import run_bass_kernel_spmd

N_CORES = 8
B, A, CI, CO, E = 2048, 128, 128, 128, 4
P = 128
CAP = 4096          # default per-core per-expert token capacity
NCHUNK = 512        # max tokens per matmul (= one PSUM bank of f32)

F32 = mybir.dt.float32
BF16 = mybir.dt.bfloat16


def _split(n, step):
    """[(off, len), ...] covering n in steps of `step`."""
    return [(o, min(step, n - o)) for o in range(0, n, step)]


def build_nc(
    caps=(CAP,) * E,
    bias_vals=None,
    n_cores=N_CORES,
    loop_n=None,
    xs_tok=4096,
    n_act=0,
    blk_pat=None,
    group_n=2,
    nchunk=NCHUNK,
    psum_bufs=4,
    x_bufs=3,
    o_bufs=3,
    out_eng="sync",
    in_eng="sync",
    ablate=None,
):
    """Emit + compile the per-core kernel.

    caps: tokens per expert on this core.
    xs_tok: max tokens per input/output DMA block.
    n_act: of every 8 relu bounces, how many go to ACT (rest DVE).
    blk_pat: per-BLOCK bounce engine pattern, e.g. "vvaa" = blocks 0,1
        on DVE, blocks 2,3 on ACT (repeating).  Overrides n_act.  Whole
        blocks per engine avoid false WAW deps on shared ob tiles.
    group_n: matmuls per PSUM tile [P, group_n, nchunk]; the whole group
        is bounced to SBUF with a single DVE/ACT op.
    nchunk: tokens per matmul (512 = one PSUM bank of f32; bf16 moving
        operand supports up to 1024).
    loop_n: wrap the compute loop in a hardware For loop (timing only).
    ablate: None | 'dma' (skip compute) | 'noout' (skip out-DMA) |
        'noin' (compute from static tile, skip in-DMA) |
        'nodma' (compute only: static input, no out-DMA) |
        'empty' (loop body is one tiny memset: For_i barrier cost) |
        'pe' (in-DMA + matmuls + out-DMA of static; no PSUM drains) |
        'drain' (in-DMA + bounces from a pre-filled PSUM tile + out-DMA;
        one matmul per block).
    """
    total = int(sum(caps))
    use_bias = bias_vals is not None and np.any(bias_vals != 0)

    nc = bacc.Bacc(
        "TRN2", target_bir_lowering=False, debug=False, num_devices=n_cores
    )
    out_dma = {"sync": nc.sync, "scalar": nc.scalar, "gpsimd": nc.gpsimd}[out_eng]
    in_dma = {"sync": nc.sync, "scalar": nc.scalar, "gpsimd": nc.gpsimd}[in_eng]
    xt_d = nc.dram_tensor("xt", [P, total], BF16, kind="ExternalInput").ap()
    w_d = nc.dram_tensor("w", [P, E, CO], BF16, kind="ExternalInput").ap()
    out_d = nc.dram_tensor("out", [P, total], BF16, kind="ExternalOutput").ap()

    with tile.TileContext(nc) as tc:
        with tc.tile_pool(name="const", bufs=1) as cpool:
            w_sb = cpool.tile([P, E, CO], BF16)
            nc.sync.dma_start(out=w_sb, in_=w_d)
            if use_bias:
                # bias columns [o, e] f32 (per-partition scalar per expert)
                b_cols = np.ascontiguousarray(
                    np.asarray(bias_vals, np.float32).T
                )
                b_const = nc.inline_tensor(b_cols, "bias").ap()
                b_sb = cpool.tile([P, E], F32)
                nc.sync.dma_start(out=b_sb, in_=b_const)
            if ablate in (
                "dma", "noin", "nodma", "pe", "drain", "pestatic", "pehalf"
            ):
                static_sb = cpool.tile([P, xs_tok], BF16)
                nc.vector.memset(static_sb, 0.25)
            if ablate == "empty":
                tiny = cpool.tile([P, 8], F32)

            with (
                tc.tile_pool(name="xin", bufs=x_bufs) as xpool,
                tc.tile_pool(name="ps", bufs=psum_bufs, space="PSUM") as pspool,
                tc.tile_pool(name="outp", bufs=o_bufs) as opool,
            ):
                loop_cm = (
                    tc.For_i(0, loop_n, 1) if loop_n else contextlib.nullcontext()
                )
                with loop_cm:
                    if ablate == "empty":
                        nc.vector.memset(tiny, 0.0)
                    n_bounce = 0
                    n_blk = 0
                    eoff = 0
                    for e in (range(E) if ablate != "empty" else []):
                        for boff_, blen in _split(caps[e], xs_tok):
                            boff = eoff + boff_
                            blk_act = (
                                blk_pat is not None
                                and blk_pat[n_blk % len(blk_pat)] == "a"
                            )
                            n_blk += 1
                            if ablate not in ("dma", "noin", "nodma"):
                                xb = xpool.tile([P, xs_tok], BF16, tag="xb")
                                in_dma.dma_start(
                                    out=xb[:, :blen],
                                    in_=xt_d[:, boff : boff + blen],
                                )
                            else:
                                xb = static_sb
                            ob = opool.tile([P, xs_tok], BF16, tag="ob")
                            if ablate != "dma":
                                # group full-size chunks group_n at a time
                                chunks = _split(blen, nchunk)
                                groups = []
                                i = 0
                                while i < len(chunks):
                                    j = i
                                    while (
                                        j < min(i + group_n, len(chunks))
                                        and chunks[j][1] == nchunk
                                    ):
                                        j += 1
                                    if j == i:
                                        j = i + 1  # lone tail chunk
                                    groups.append(chunks[i:j])
                                    i = j
                                ps_blk = None
                                if ablate == "drain":
                                    ps_blk = pspool.tile(
                                        [P, group_n, nchunk], F32, tag="ps"
                                    )
                                    for j in range(group_n):
                                        nc.tensor.matmul(
                                            ps_blk[:, j],
                                            w_sb[:, e],
                                            xb[:, :nchunk],
                                            start=True,
                                            stop=True,
                                        )
                                for gi, grp in enumerate(groups):
                                    ng = len(grp)
                                    if ablate == "pehalf" and gi % 2 == 1:
                                        continue
                                    if ablate == "drain":
                                        ps = ps_blk
                                    else:
                                        src = (
                                            static_sb
                                            if ablate == "pestatic"
                                            else xb
                                        )
                                        ps = pspool.tile(
                                            [P, group_n, nchunk], F32, tag="ps"
                                        )
                                        for j, (coff, clen) in enumerate(grp):
                                            nc.tensor.matmul(
                                                ps[:, j, :clen],
                                                w_sb[:, e],
                                                src[:, coff : coff + clen],
                                                start=True,
                                                stop=True,
                                            )
                                    if ablate in ("pe", "pestatic", "pehalf"):
                                        continue
                                    g0 = grp[0][0]
                                    glen = sum(c[1] for c in grp)
                                    osl = ob[:, g0 : g0 + glen]
                                    if ng > 1:
                                        psl = ps[:, :ng].rearrange(
                                            "p a b -> p (a b)"
                                        )
                                    else:
                                        psl = ps[:, 0, : grp[0][1]]
                                    if blk_pat is not None:
                                        on_act = blk_act
                                    else:
                                        on_act = (n_bounce % 8) < n_act
                                    n_bounce += 1
                                    if use_bias:
                                        if on_act:
                                            nc.scalar.activation(
                                                osl,
                                                psl,
                                                mybir.ActivationFunctionType.Relu,
                                                bias=b_sb[:, e : e + 1],
                                            )
                                        else:
                                            nc.vector.tensor_scalar(
                                                osl,
                                                psl,
                                                b_sb[:, e : e + 1],
                                                0.0,
                                                mybir.AluOpType.add,
                                                mybir.AluOpType.max,
                                            )
                                    else:
                                        if on_act:
                                            nc.scalar.activation(
                                                osl,
                                                psl,
                                                mybir.ActivationFunctionType.Relu,
                                            )
                                        else:
                                            nc.vector.tensor_scalar(
                                                osl,
                                                psl,
                                                0.0,
                                                None,
                                                mybir.AluOpType.max,
                                            )
                            else:
                                nc.vector.tensor_copy(
                                    out=ob[:, :blen], in_=static_sb[:, :blen]
                                )
                            if ablate not in ("noout", "nodma"):
                                out_dma.dma_start(
                                    out=out_d[:, boff : boff + blen],
                                    in_=(
                                        static_sb[:, :blen]
                                        if ablate in ("pe", "pestatic", "pehalf")
                                        else ob[:, :blen]
                                    ),
                                )
                        eoff += caps[e]

    nc.compile()
    return nc


def build_nc_dual(
    caps=(CAP,) * E,
    bias_vals=None,
    n_cores=N_CORES,
    loop_n=None,
    xs_tok=4096,
    group_n=2,
    nchunk=NCHUNK,
    psum_bufs=2,
    x_bufs=2,
    o_bufs=2,
    in_eng="sync",
    out_eng="gpsimd",
    lanes=((0, 1), (2, 3)),
    edge_split=0,
    head_split=0,
    tail_split=0,
    warm_mm=0,
    out_per_group=False,
    tail_halves=False,
):
    """Dual-lane kernel: lane 0 bounces on DVE, lane 1 on ACT.

    tail_halves: the LAST block of each lane stores its output as two
    half-size DMAs (the first fires while the block's later groups are
    still draining), shortening the critical out tail.

    warm_mm: emit this many dummy matmuls (reading w_sb, writing a
    scratch slot of lane 0's PSUM pool) at the top of each pass so the
    PE's p-state ramp (~3us of continuous activity -> 2.4 GHz) completes
    during the first input DMA instead of eating into real matmul time.

    (Note: staggering the k>=1 in-DMA triggers via nc.sync.nop(cycle_cnt)
    to give block 0 full DMA bandwidth is NOT possible — that nop lowers
    to ISA opcode 164, unimplemented in the Tile scheduler simulator.)

    Each lane has its own xb/psum/ob pools and processes its experts'
    blocks; lanes interleave at group granularity in program order so
    the PE alternates between DVE-drained and ACT-drained PSUM groups
    and both drain engines run concurrently.
    """
    total = int(sum(caps))
    use_bias = bias_vals is not None and np.any(bias_vals != 0)

    nc = bacc.Bacc(
        "TRN2", target_bir_lowering=False, debug=False, num_devices=n_cores
    )
    engs = {"sync": nc.sync, "scalar": nc.scalar, "gpsimd": nc.gpsimd}
    in_dma = engs[in_eng]
    out_dma = engs[out_eng]

    xt_d = nc.dram_tensor("xt", [P, total], BF16, kind="ExternalInput").ap()
    w_d = nc.dram_tensor("w", [P, E, CO], BF16, kind="ExternalInput").ap()
    out_d = nc.dram_tensor("out", [P, total], BF16, kind="ExternalOutput").ap()

    eoffs = np.concatenate([[0], np.cumsum(caps)]).astype(int)
    # lane -> list of (expert, dram_off, blen).  edge_split carves a
    # small first block (first expert) and small last block (last
    # expert) so the pass's entry/exit DMAs are short.
    hs = head_split or edge_split
    ts = tail_split or edge_split
    lane_blocks = []
    for lane in lanes:
        blocks = []
        for ei, e in enumerate(lane):
            sizes = []
            rem = caps[e]
            head = tail = 0
            if hs and ei == 0 and rem > hs:
                head = hs
                rem -= head
            if ts and ei == len(lane) - 1 and rem > ts:
                tail = ts
                rem -= tail
            if head:
                sizes.append(head)
            sizes += [s for _, s in _split(rem, xs_tok)] if rem else []
            if tail:
                sizes.append(tail)
            off = 0
            for s in sizes:
                blocks.append((e, int(eoffs[e]) + off, s))
                off += s
        lane_blocks.append(blocks)
    n_blk_max = max(len(bl) for bl in lane_blocks)

    with tile.TileContext(nc) as tc:
        with tc.tile_pool(name="const", bufs=1) as cpool:
            w_sb = cpool.tile([P, E, CO], BF16)
            nc.sync.dma_start(out=w_sb, in_=w_d)
            if use_bias:
                b_cols = np.ascontiguousarray(
                    np.asarray(bias_vals, np.float32).T
                )
                b_const = nc.inline_tensor(b_cols, "bias").ap()
                b_sb = cpool.tile([P, E], F32)
                nc.sync.dma_start(out=b_sb, in_=b_const)

            import contextlib as _ctx

            with _ctx.ExitStack() as stack:
                xpools, pspools, opools = [], [], []
                for li in range(len(lanes)):
                    xpools.append(stack.enter_context(
                        tc.tile_pool(name=f"xin{li}", bufs=x_bufs)))
                    pspools.append(stack.enter_context(
                        tc.tile_pool(name=f"ps{li}", bufs=psum_bufs,
                                     space="PSUM")))
                    opools.append(stack.enter_context(
                        tc.tile_pool(name=f"outp{li}", bufs=o_bufs)))

                loop_cm = (
                    tc.For_i(0, loop_n, 1) if loop_n else _ctx.nullcontext()
                )
                with loop_cm:
                    if warm_mm:
                        wps = pspools[0].tile(
                            [P, group_n, nchunk], F32, tag="ps"
                        )
                        w_flat = w_sb.rearrange("p e o -> p (e o)")
                        for i in range(warm_mm):
                            nc.tensor.matmul(
                                wps[:, i % group_n],
                                w_sb[:, i % E],
                                w_flat[:, :nchunk],
                                start=True,
                                stop=True,
                            )
                    for k in range(n_blk_max):
                        cur = []  # per-lane (blk, xb, ob, groups)
                        for li, blocks in enumerate(lane_blocks):
                            if k >= len(blocks):
                                cur.append(None)
                                continue
                            e, boff, blen = blocks[k]
                            xb = xpools[li].tile([P, xs_tok], BF16, tag="xb")
                            in_dma.dma_start(
                                out=xb[:, :blen],
                                in_=xt_d[:, boff : boff + blen],
                            )
                            is_tail = (
                                tail_halves
                                and k == len(lane_blocks[li]) - 1
                                and blen % 2 == 0
                            )
                            if out_per_group:
                                ob = None
                            elif is_tail:
                                ob = [
                                    opools[li].tile(
                                        [P, xs_tok], BF16, tag="ob"
                                    )
                                    for _ in range(2)
                                ]
                            else:
                                ob = opools[li].tile(
                                    [P, xs_tok], BF16, tag="ob"
                                )
                            chunks = _split(blen, nchunk)
                            groups = []
                            i = 0
                            while i < len(chunks):
                                j = i
                                while (
                                    j < min(i + group_n, len(chunks))
                                    and chunks[j][1] == nchunk
                                ):
                                    j += 1
                                if j == i:
                                    j = i + 1
                                groups.append(chunks[i:j])
                                i = j
                            cur.append(
                                (e, boff, blen, xb, ob, groups, is_tail)
                            )
                        n_grp_max = max(
                            len(c[5]) for c in cur if c is not None
                        )
                        for g in range(n_grp_max):
                            for li, c in enumerate(cur):
                                if c is None or g >= len(c[5]):
                                    continue
                                e, boff, blen, xb, ob, groups, is_tail = c
                                grp = groups[g]
                                ng = len(grp)
                                ps = pspools[li].tile(
                                    [P, group_n, nchunk], F32, tag="ps"
                                )
                                for j, (coff, clen) in enumerate(grp):
                                    nc.tensor.matmul(
                                        ps[:, j, :clen],
                                        w_sb[:, e],
                                        xb[:, coff : coff + clen],
                                        start=True,
                                        stop=True,
                                    )
                                g0 = grp[0][0]
                                glen = sum(x[1] for x in grp)
                                if out_per_group:
                                    ob_g = opools[li].tile(
                                        [P, group_n * nchunk], BF16, tag="ob"
                                    )
                                    osl = ob_g[:, :glen]
                                elif is_tail:
                                    half = blen // 2
                                    hh = 1 if g0 >= half else 0
                                    osl = ob[hh][
                                        :, g0 - hh * half : g0 - hh * half + glen
                                    ]
                                else:
                                    osl = ob[:, g0 : g0 + glen]
                                if ng > 1:
                                    psl = ps[:, :ng].rearrange(
                                        "p a b -> p (a b)"
                                    )
                                else:
                                    psl = ps[:, 0, : grp[0][1]]
                                if li == 0:
                                    if use_bias:
                                        nc.vector.tensor_scalar(
                                            osl, psl, b_sb[:, e : e + 1],
                                            0.0, mybir.AluOpType.add,
                                            mybir.AluOpType.max,
                                        )
                                    else:
                                        nc.vector.tensor_scalar(
                                            osl, psl, 0.0, None,
                                            mybir.AluOpType.max,
                                        )
                                else:
                                    if use_bias:
                                        nc.scalar.activation(
                                            osl, psl,
                                            mybir.ActivationFunctionType.Relu,
                                            bias=b_sb[:, e : e + 1],
                                        )
                                    else:
                                        nc.scalar.activation(
                                            osl, psl,
                                            mybir.ActivationFunctionType.Relu,
                                        )
                                if out_per_group:
                                    out_dma.dma_start(
                                        out=out_d[
                                            :, boff + g0 : boff + g0 + glen
                                        ],
                                        in_=ob_g[:, :glen],
                                    )
                                elif is_tail and g0 + glen in (
                                    blen // 2,
                                    blen,
                                ):
                                    half = blen // 2
                                    hh = 1 if g0 >= half else 0
                                    out_dma.dma_start(
                                        out=out_d[
                                            :,
                                            boff + hh * half :
                                            boff + hh * half + half,
                                        ],
                                        in_=ob[hh][:, :half],
                                    )
                        if not out_per_group:
                            for li, c in enumerate(cur):
                                if c is None:
                                    continue
                                e, boff, blen, xb, ob, groups, is_tail = c
                                if is_tail:
                                    continue
                                out_dma.dma_start(
                                    out=out_d[:, boff : boff + blen],
                                    in_=ob[:, :blen],
                                )

    nc.compile()
    return nc


_NC_CACHE = {}

# Best measured configuration (applied to the graded kernel() path and
# to test.py's timing build): dual-lane (DVE+ACT bounce split) with PE
# p-state warmup matmuls during the input-DMA edge.
DEFAULT_BUILD_KWARGS = {"_dual": True, "warm_mm": 10}


def _get_nc(caps, bias_key, bias_vals):
    key = (tuple(caps), bias_key)
    if key not in _NC_CACHE:
        import time

        t0 = time.time()
        kw = dict(DEFAULT_BUILD_KWARGS)
        builder = build_nc_dual if kw.pop("_dual", False) else build_nc
        _NC_CACHE[key] = builder(caps, bias_vals, **kw)
        print(f"[kernel] build_nc: {time.time() - t0:.1f}s", file=sys.stderr)
    return _NC_CACHE[key]


def prepare_inputs(x, atom_numbers, W, b, cases, caps=(CAP,) * E):
    """Host-side prep: bucket by expert, gather, transpose, cast.

    Returns (in_maps, gidx, valid) where gidx/valid are [N_CORES, total]
    arrays mapping device slots back to flat token indices.
    """
    import ml_dtypes

    total = int(sum(caps))
    an_flat = np.ascontiguousarray(atom_numbers, dtype=np.int32).reshape(-1)
    x_flat = np.ascontiguousarray(x, dtype=np.float32).reshape(-1, CI)

    gidx = np.zeros((N_CORES, total), dtype=np.int64)
    valid = np.zeros((N_CORES, total), dtype=bool)
    off = 0
    for e in range(E):
        idx_e = np.nonzero(an_flat == cases[e])[0]
        cap_g = N_CORES * caps[e]
        if idx_e.size > cap_g:
            raise OverflowError(
                f"expert {e}: {idx_e.size} tokens > capacity {cap_g}"
            )
        padded = np.zeros(cap_g, dtype=np.int64)
        padded[: idx_e.size] = idx_e
        vmask = np.zeros(cap_g, dtype=bool)
        vmask[: idx_e.size] = True
        gidx[:, off : off + caps[e]] = padded.reshape(N_CORES, caps[e])
        valid[:, off : off + caps[e]] = vmask.reshape(N_CORES, caps[e])
        off += caps[e]

    x_bf = x_flat.astype(ml_dtypes.bfloat16)
    # [N_CORES, total, CI] -> [N_CORES, CI, total]
    xg = x_bf[gidx.reshape(-1)].reshape(N_CORES, total, CI)
    xt = np.ascontiguousarray(xg.transpose(0, 2, 1))

    w_t = np.ascontiguousarray(
        np.asarray(W, np.float32).transpose(1, 0, 2)
    ).astype(ml_dtypes.bfloat16)

    in_maps = [{"xt": xt[c], "w": w_t} for c in range(N_CORES)]
    return in_maps, gidx, valid


def _kernel_numpy(x, atom_numbers, W, b, cases):
    """Exact fallback (duplicate cases / capacity overflow)."""
    x = np.asarray(x, np.float32)
    an = np.asarray(atom_numbers)
    W = np.asarray(W, np.float32)
    b = np.asarray(b, np.float32)
    Bb, Aa, Ci = x.shape
    xf = x.reshape(-1, Ci)
    anf = an.reshape(-1)
    out = np.zeros((Bb * Aa, W.shape[2]), np.float32)
    for e in range(W.shape[0]):
        sel = anf == cases[e]
        if np.any(sel):
            out[sel] += np.maximum(xf[sel] @ W[e] + b[e], 0.0)
    return out.reshape(Bb, Aa, W.shape[2])


def kernel(x, atom_numbers, W, b, atom_cases):
    x = np.asarray(x)
    cases = [int(v) for v in np.asarray(atom_cases).reshape(-1)]
    b_np = np.asarray(b, dtype=np.float32)

    Bf, Af, CIf = x.shape
    assert (Bf, Af, CIf) == (B, A, CI), (Bf, Af, CIf)
    if len(set(cases)) != len(cases):
        return _kernel_numpy(x, atom_numbers, W, b_np, cases)

    caps = (CAP,) * E
    try:
        in_maps, gidx, valid = prepare_inputs(
            x, atom_numbers, W, b_np, cases, caps
        )
    except OverflowError:
        return _kernel_numpy(x, atom_numbers, W, b_np, cases)

    bias_key = bool(np.any(b_np != 0))
    nc = _get_nc(caps, bias_key, b_np if bias_key else None)

    res = run_bass_kernel_spmd(nc, in_maps, list(range(N_CORES)))
    total = int(sum(caps))
    # [cores][o, tok] -> [cores*total, o]
    out_rows = (
        np.stack([np.asarray(r["out"]) for r in res.results], axis=0)
        .transpose(0, 2, 1)
        .reshape(N_CORES * total, CO)
        .astype(np.float32)
    )
    vflat = valid.reshape(-1)
    out_full = np.zeros((B * A, CO), dtype=np.float32)
    out_full[gidx.reshape(-1)[vflat]] = out_rows[vflat]
    return out_full.reshape(B, A, CO)
